# revision 1
# baseline (speedup 1.0000x reference)
"""Trainium2 Bass kernel for BasicEuclideanDistModel (gnn_message_passing).

Math:
  result = sum_e (beta - ||dz_e + dv_e t_e||)
           - dt * sum_{i<j, s} exp(beta - ||z_i(t_s) - z_j(t_s)||)

Device strategy (8 cores, data parallel):
  * Non-event term: full NxN pairwise distances (halved on host).
    d^2(i,j,s) = r_i(s) + r_j(s) - 2 x_i(s)x_j(s) - 2 y_i(s)y_j(s)
    decomposes into a K=8 inner product  F_i(s) . G_j  where G_j is
    time-INdependent:
      F_i(s) = [r_i(s), 1, t_s, t_s^2, -2x_i(s), -2t_s x_i(s), -2y_i(s), -2t_s y_i(s)]
      G_j    = [1,  a_j, b_j, c_j,  zx_j, vx_j, zy_j, vy_j]
    with r(s) = a + b t + c t^2, a = zx^2+zy^2, b = 2(zx vx + zy vy),
    c = vx^2+vy^2.  One [8,128]x[8,2048] matmul (fp32r) per (i-tile, s)
    computes the d^2 supertile; DVE relu clamps rounding negatives,
    ACT computes sqrt then exp(-d) with fused per-partition row sums.
    Each core owns 2 of the 16 i-tiles (rows), all j, all 10 samples.
  * Event term: 25000 events/core. Events are grouped by u-node into
    8-slot segments: the u-side row is dma_gather'ed once per segment
    (4.6k descriptors) and expanded with step-0 broadcast views; the
    v-side is dma_gather'ed per slot across 4 SWDGE queues. Pad slots
    use v=node, t=0 so they contribute exactly 0. DVE distance algebra
    runs after the main loop (in-order engine streams), ACT sqrt with
    fused row-sum finishes the term.
  * beta enters only as a scalar factor / offset -> folded in on host:
    sum exp(beta - d) = e^beta * sum exp(-d);  sum(beta - d) = E*beta - sum d.
  Host combines 8 cores' [128, 24] partial-sum tensors (pure unshard/
  reduction of partials).
"""

import os
import numpy as np


def _import_concourse():
    try:
        import concourse  # noqa: F401
    except ImportError:
        import sys

        for p in ("/opt/trn_rl_repo", "/root/.axon_site/_ro/trn_rl_repo"):
            if os.path.isdir(p) and p not in sys.path:
                sys.path.insert(0, p)


_import_concourse()

from contextlib import ExitStack  # noqa: E402

import concourse.bacc as bacc  # noqa: E402
import concourse.bass as bass  # noqa: E402
import concourse.mybir as mybir  # noqa: E402
import concourse.tile as tile  # noqa: E402
from concourse.tile_rust import add_dep_helper  # noqa: E402

N = 2048          # nodes
S = 10            # Riemann samples
NCORES = 8
ITILES = 2        # 128-row i-tiles per core
EV_PER_CORE = 200000 // NCORES       # real events per core
# Event layout: events grouped by u-node into segments of SLOT slots; the
# u-side row is gathered once per SEGMENT and broadcast across its slots.
SLOT = 8          # event slots per segment
SP = 36           # segments per partition
C_EV = SP * SLOT                     # 288 event columns per partition
NSEG = 128 * SP                      # 4608 segments per core
SEG_OPS = 4       # seg-gather split into this many dma_gather ops
SEG_PER_OP = NSEG // SEG_OPS         # 1152
EV_CHUNKS = 4     # v-side gather ops per core (one per SWDGE queue)
EV_CC = C_EV // EV_CHUNKS            # 72 event columns per chunk
EV_PER_CHUNK = 128 * EV_CC           # 9216
GELEM = 64        # gather element size in f32 (256B rows, dma_gather minimum)

F32 = mybir.dt.float32
F32R = mybir.dt.float32r
BF16 = mybir.dt.bfloat16
I32 = mybir.dt.int32
I16 = mybir.dt.int16
AF = mybir.ActivationFunctionType
OP = mybir.AluOpType

_CACHE: dict = {}


def _tt(nc, out, in0, in1, op):
    return nc.vector.tensor_tensor(out, in0, in1, op=op)


def _build():
    if "nc" in _CACHE:
        return _CACHE["nc"]

    nc = bacc.Bacc(
        "TRN2", target_bir_lowering=False, debug=False, enable_asserts=False,
        num_swdge_queues=4,
    )

    zv_all = nc.dram_tensor("zv_all", [N, 4], F32, kind="ExternalInput").ap()
    zv_pad = nc.dram_tensor("zv_pad", [N, GELEM], F32, kind="ExternalInput").ap()
    zv_i = nc.dram_tensor("zv_i", [ITILES * 128, 4], F32, kind="ExternalInput").ap()
    # int16 indices, dma_gather wrap: op ch's index k lives at
    # [k % 16, ch, k // 16], replicated down all 8 blocks of 16 partitions
    ev_u = nc.dram_tensor(
        "ev_u", [128, SEG_OPS, SEG_PER_OP // 16], I16, kind="ExternalInput"
    ).ap()
    ev_v = nc.dram_tensor(
        "ev_v", [128, EV_CHUNKS, EV_PER_CHUNK // 16], I16, kind="ExternalInput"
    ).ap()
    ev_t = nc.dram_tensor("ev_t", [128, C_EV], F32, kind="ExternalInput").ap()
    tb_d = nc.dram_tensor("tb", [128, S], F32, kind="ExternalInput").ap()
    t2b_d = nc.dram_tensor("t2b", [128, S], F32, kind="ExternalInput").ap()
    ident_d = nc.dram_tensor("ident", [128, 128], F32, kind="ExternalInput").ap()
    out_p = nc.dram_tensor("out_p", [128, 24], F32, kind="ExternalOutput").ap()

    with tile.TileContext(nc) as tc, ExitStack() as ctx:
        cpool = ctx.enter_context(tc.tile_pool(name="const", bufs=1))
        evpool = ctx.enter_context(tc.tile_pool(name="ev", bufs=1))

        # ---------------- input loads ----------------
        # event index loads first: the Pool engine spends the whole kernel
        # generating gather descriptors, so the gathers must start ASAP
        # per-op index loads: gather k waits only for its own small slice
        u_sb = evpool.tile([128, SEG_OPS, SEG_PER_OP // 16], I16)
        for so in range(SEG_OPS):
            nc.sync.dma_start(u_sb[:, so, :], ev_u[:, so, :])
        v_sb = evpool.tile([128, EV_CHUNKS, EV_PER_CHUNK // 16], I16)
        for ch in range(EV_CHUNKS):
            nc.sync.dma_start(v_sb[:, ch, :], ev_v[:, ch, :])
        t_sb = evpool.tile([128, C_EV], F32)
        nc.sync.dma_start(t_sb[:], ev_t)
        zv_sb = cpool.tile([128, 16, 4], F32)        # all nodes, j-side
        nc.sync.dma_start(zv_sb[:], zv_all.rearrange("(c p) d -> p c d", p=128))
        zvi_sb = cpool.tile([128, ITILES, 4], F32)   # this core's i rows
        nc.sync.dma_start(zvi_sb[:], zv_i.rearrange("(c p) d -> p c d", p=128))
        tb = cpool.tile([128, S], F32)
        nc.sync.dma_start(tb[:], tb_d)
        t2b = cpool.tile([128, S], F32)
        nc.sync.dma_start(t2b[:], t2b_d)
        # identity comes from the host: building it with make_identity would
        # occupy the gpsimd engine ahead of the gather descriptor generation
        ident = cpool.tile([128, 128], F32)
        nc.sync.dma_start(ident[:], ident_d)

        acc = cpool.tile([128, 24], F32)
        nc.vector.memset(acc[:], 0.0)

        # ---------------- event gathers + distance algebra ----------------
        # u-side: one 256B row per SEGMENT into seg[128, SP, GELEM], expanded
        # over the segment's SLOT event slots via step-0 broadcast views.
        # v-side: one row per event slot (pads gather v=node, t=0 -> d=0).
        d2all = evpool.tile([128, C_EV, 1], F32)
        seg = evpool.tile([128, SP, GELEM], F32)
        for so in range(SEG_OPS):
            nc.gpsimd.dma_gather(
                seg[:, so * (SP // SEG_OPS):(so + 1) * (SP // SEG_OPS), :],
                zv_pad, u_sb[:, so, :], SEG_PER_OP, SEG_PER_OP, GELEM,
                single_packet=False, queue_num=so % 4,
            )
        SEG_CC = EV_CC // SLOT  # segments covered by one v-chunk
        # issue all v gathers upfront (one per SWDGE queue); the distance
        # algebra runs AFTER the main loop so the in-order DVE stream is not
        # blocked waiting on gather completions
        evg = ctx.enter_context(tc.tile_pool(name="evg", bufs=4))
        b_tiles = []
        for ch in range(EV_CHUNKS):
            B = evg.tile([128, EV_CC, GELEM], F32, tag="B", name="B")
            nc.gpsimd.dma_gather(
                B[:], zv_pad, v_sb[:, ch, :], EV_PER_CHUNK, EV_PER_CHUNK, GELEM,
                single_packet=False, queue_num=ch % 4,
            )
            b_tiles.append(B)

        def emit_event_math(ch, scratch_pool):
            B = b_tiles[ch]
            q0 = ch * SEG_CC
            shape4 = [128, SEG_CC, SLOT, 1]
            tse = (
                t_sb[:, ch * EV_CC:(ch + 1) * EV_CC]
                .rearrange("p (q j) -> p q j", j=SLOT)
                .unsqueeze(3)
            )

            def sv(d):  # seg channel d view broadcast over the slots
                return (
                    seg[:, q0:q0 + SEG_CC, d:d + 1]
                    .unsqueeze(2)
                    .to_broadcast(shape4)
                )

            def bv(d):  # B channel d view
                return B[:, :, d:d + 1].rearrange("p (q j) d -> p q j d", j=SLOT)

            # scratch from the main-loop w pool (same tag): the slot-reuse
            # WAR deps place these after the main loop in the DVE stream
            dzx = scratch_pool.tile(shape4, F32, tag="w", name="dzx")
            dvx = scratch_pool.tile(shape4, F32, tag="w", name="dvx")
            dzy = scratch_pool.tile(shape4, F32, tag="w", name="dzy")
            dvy = scratch_pool.tile(shape4, F32, tag="w", name="dvy")
            first = _tt(nc, dzx[:], sv(0), bv(0), OP.subtract)
            _tt(nc, dvx[:], sv(2), bv(2), OP.subtract)
            _tt(nc, dvx[:], dvx[:], tse, OP.mult)
            _tt(nc, dzx[:], dzx[:], dvx[:], OP.add)          # dx
            _tt(nc, dzy[:], sv(1), bv(1), OP.subtract)
            _tt(nc, dvy[:], sv(3), bv(3), OP.subtract)
            _tt(nc, dvy[:], dvy[:], tse, OP.mult)
            _tt(nc, dzy[:], dzy[:], dvy[:], OP.add)          # dy
            _tt(nc, dzx[:], dzx[:], dzx[:], OP.mult)
            _tt(nc, dzy[:], dzy[:], dzy[:], OP.mult)
            d2v = d2all[:, ch * EV_CC:(ch + 1) * EV_CC, :].rearrange(
                "p (q j) d -> p q j d", j=SLOT
            )
            _tt(nc, d2v, dzx[:], dzy[:], OP.add)             # d^2
            return first

        # ---------------- j features  F[p, chunk, 0:8] ----------------
        # [1, a, b, c, zx, vx, zy, vy]; padded to 32 for the PE transpose
        F = cpool.tile([128, 16, 32], F32)
        zx = zv_sb[:, :, 0:1]
        zy = zv_sb[:, :, 1:2]
        vx = zv_sb[:, :, 2:3]
        vy = zv_sb[:, :, 3:4]
        s1 = cpool.tile([128, 16, 1], F32)
        nc.vector.memset(F[:, :, 0:1], 1.0)
        _tt(nc, F[:, :, 1:2], zx, zx, OP.mult)           # a = zx^2 + zy^2
        _tt(nc, s1[:], zy, zy, OP.mult)
        _tt(nc, F[:, :, 1:2], F[:, :, 1:2], s1[:], OP.add)
        s2 = cpool.tile([128, 16, 1], F32)
        _tt(nc, F[:, :, 2:3], zx, vx, OP.mult)           # b = 2(zx vx + zy vy)
        _tt(nc, s2[:], zy, vy, OP.mult)
        _tt(nc, F[:, :, 2:3], F[:, :, 2:3], s2[:], OP.add)
        nc.vector.tensor_scalar_mul(F[:, :, 2:3], F[:, :, 2:3], 2.0)
        s3 = cpool.tile([128, 16, 1], F32)
        _tt(nc, F[:, :, 3:4], vx, vx, OP.mult)           # c = vx^2 + vy^2
        _tt(nc, s3[:], vy, vy, OP.mult)
        _tt(nc, F[:, :, 3:4], F[:, :, 3:4], s3[:], OP.add)
        nc.vector.tensor_copy(F[:, :, 4:5], zx)
        nc.vector.tensor_copy(F[:, :, 5:6], vx)
        nc.vector.tensor_copy(F[:, :, 6:7], zy)
        nc.vector.tensor_copy(F[:, :, 7:8], vy)

        # ---------------- i features  L[p, it, s, 0:8] ----------------
        # [r, 1, t, t^2, -2x, -2tx, -2y, -2ty]
        L = cpool.tile([128, ITILES, S, 32], F32)
        izx = zvi_sb[:, :, 0:1]
        izy = zvi_sb[:, :, 1:2]
        ivx = zvi_sb[:, :, 2:3]
        ivy = zvi_sb[:, :, 3:4]
        # a, b, c for the i rows: [128, ITILES, 1]
        ia = cpool.tile([128, ITILES, 1], F32)
        ib = cpool.tile([128, ITILES, 1], F32)
        ic = cpool.tile([128, ITILES, 1], F32)
        s4 = cpool.tile([128, ITILES, 1], F32)
        _tt(nc, ia[:], izx, izx, OP.mult)
        _tt(nc, s4[:], izy, izy, OP.mult)
        _tt(nc, ia[:], ia[:], s4[:], OP.add)
        s5 = cpool.tile([128, ITILES, 1], F32)
        _tt(nc, ib[:], izx, ivx, OP.mult)
        _tt(nc, s5[:], izy, ivy, OP.mult)
        _tt(nc, ib[:], ib[:], s5[:], OP.add)
        nc.vector.tensor_scalar_mul(ib[:], ib[:], 2.0)
        s6 = cpool.tile([128, ITILES, 1], F32)
        _tt(nc, ic[:], ivx, ivx, OP.mult)
        _tt(nc, s6[:], ivy, ivy, OP.mult)
        _tt(nc, ic[:], ic[:], s6[:], OP.add)

        def b_i(v):  # [128, ITILES, 1] -> [128, ITILES, S, 1]
            return v.unsqueeze(2).to_broadcast([128, ITILES, S, 1])

        tv = tb.unsqueeze(1).unsqueeze(3).to_broadcast([128, ITILES, S, 1])
        t2v = t2b.unsqueeze(1).unsqueeze(3).to_broadcast([128, ITILES, S, 1])

        nc.vector.memset(L[:, :, :, 1:2], 1.0)
        nc.vector.tensor_copy(L[:, :, :, 2:3], tv)
        nc.vector.tensor_copy(L[:, :, :, 3:4], t2v)
        Lx = cpool.tile([128, ITILES, S, 1], F32)
        _tt(nc, Lx[:], b_i(ivx), tv, OP.mult)            # x_i(s) = zx + vx t
        _tt(nc, Lx[:], Lx[:], b_i(izx), OP.add)
        nc.vector.tensor_scalar_mul(L[:, :, :, 4:5], Lx[:], -2.0)
        _tt(nc, L[:, :, :, 5:6], L[:, :, :, 4:5], tv, OP.mult)
        Ly = cpool.tile([128, ITILES, S, 1], F32)
        _tt(nc, Ly[:], b_i(ivy), tv, OP.mult)
        _tt(nc, Ly[:], Ly[:], b_i(izy), OP.add)
        nc.vector.tensor_scalar_mul(L[:, :, :, 6:7], Ly[:], -2.0)
        _tt(nc, L[:, :, :, 7:8], L[:, :, :, 6:7], tv, OP.mult)
        Lr = cpool.tile([128, ITILES, S, 1], F32)
        _tt(nc, L[:, :, :, 0:1], b_i(ib), tv, OP.mult)   # r = a + b t + c t^2
        _tt(nc, L[:, :, :, 0:1], L[:, :, :, 0:1], b_i(ia), OP.add)
        _tt(nc, Lr[:], b_i(ic), t2v, OP.mult)
        _tt(nc, L[:, :, :, 0:1], L[:, :, :, 0:1], Lr[:], OP.add)

        # ---------------- transposes (PE) ----------------
        # transpose copies write float32r directly (rounds for the fp32r
        # matmul; Bacc's generate_event_semaphores legalizes the waits)
        T2 = cpool.tile([8, N], F32R)                    # G_j rows
        L2 = cpool.tile([8, ITILES * S, 128], F32R)      # F_i(s) rows
        with tc.tile_pool(name="tp", bufs=4, space="PSUM") as tpp:
            for c in range(16):
                pt = tpp.tile([32, 128], F32, tag="pt", name="pt")
                nc.tensor.transpose(pt[:], F[:, c, :], ident[:])
                nc.vector.tensor_copy(T2[:, c * 128:(c + 1) * 128], pt[0:8, :])
            for it in range(ITILES):
                for s in range(S):
                    pt = tpp.tile([32, 128], F32, tag="pt", name="pt")
                    nc.tensor.transpose(pt[:], L[:, it, s, :], ident[:])
                    nc.vector.tensor_copy(L2[:, it * S + s, :], pt[0:8, :])

        d_ev = evpool.tile([128, C_EV, 1], F32)

        # ---------------- main pairwise loop ----------------
        sq_insts = [[] for _ in range(ITILES)]
        ex_insts = [[] for _ in range(ITILES)]
        relu_insts = []
        with tc.tile_pool(name="qp", bufs=2, space="PSUM") as qpool, \
                tc.tile_pool(name="wp", bufs=12) as wpool:
            for it in range(ITILES):
                for s in range(S):
                    q = qpool.tile([128, N], F32, tag="q", name="q")
                    for kk in range(4):
                        nc.tensor.matmul(
                            q[:, kk * 512:(kk + 1) * 512],
                            L2[:, it * S + s, :],
                            T2[:, kk * 512:(kk + 1) * 512],
                            start=True, stop=True,
                        )
                    w = wpool.tile([128, N], BF16, tag="w", name="w")
                    relu_insts.append(
                        nc.vector.tensor_scalar_max(w[:], q[:], 0.0)
                    )
                    col = it * S + s
                    sq = nc.scalar.activation(w[:], w[:], AF.Sqrt)
                    ex = nc.scalar.activation(
                        w[:], w[:], AF.Exp, scale=-1.0,
                        accum_out=acc[:, col:col + 1],
                    )
                    sq_insts[it].append(sq)
                    ex_insts[it].append(ex)

            # event distance algebra AFTER the relus in the DVE stream:
            # its inputs (gathers) complete long after the main loop's
            # DVE work is ready, and engine streams execute in order
            for ch in range(EV_CHUNKS):
                emit_event_math(ch, wpool)

            ev_sq = nc.scalar.activation(
                d_ev[:], d2all[:], AF.Sqrt, accum_out=acc[:, 20:21]
            )

            # Force ACT phase order: sqrt(i0) exp(i0) sqrt(i1) exp(i1) ev.
            # The event gathers land late, so the event sqrt goes last
            # (one extra table load, but no ACT stall).
            order = (
                sq_insts[0] + ex_insts[0] + sq_insts[1] + ex_insts[1] + [ev_sq]
            )
            for a, b in zip(order[1:], order[:-1]):
                add_dep_helper(a.ins, b.ins, reason="act table phase order")

            nc.sync.dma_start(out_p, acc[:])

    nc.compile()  # wait legalization (1 sync wait / instruction) + act table loads
    _CACHE["nc"] = nc
    return nc


def _marshal(inputs):
    z0 = np.asarray(inputs["z0"], dtype=np.float32)
    v0 = np.asarray(inputs["v0"], dtype=np.float32)
    uv = np.asarray(inputs["data_uv"], dtype=np.int32)
    tt = np.asarray(inputs["data_t"], dtype=np.float32)
    t0 = np.float32(np.asarray(inputs["t0"]).reshape(-1)[0])
    tn = np.float32(np.asarray(inputs["tn"]).reshape(-1)[0])

    zv = np.ascontiguousarray(np.concatenate([z0, v0], axis=1)).astype(np.float32)
    dt = np.float32((tn - t0) / np.float32(S))
    tmid = (t0 + (np.arange(S, dtype=np.float32) + np.float32(0.5)) * dt).astype(
        np.float32
    )
    tb = np.ascontiguousarray(np.broadcast_to(tmid, (128, S))).astype(np.float32)
    t2b = (tb * tb).astype(np.float32)

    zv_pad = np.zeros((N, GELEM), np.float32)
    zv_pad[:, 0:4] = zv

    E = uv.shape[0]
    assert E <= NCORES * EV_PER_CORE
    u_all = uv[:, 0].astype(np.int16)
    v_all = uv[:, 1].astype(np.int16)

    def pack_events(u, v, t):
        """Group a core's events by u into segments of <= SLOT slots.
        Pad slots use v=node, t=0 (distance exactly 0); unused segments
        use node 0 with v=0, t=0."""
        order = np.argsort(u, kind="stable")
        us, vs, ts = u[order], v[order], t[order]
        starts = np.flatnonzero(np.r_[True, us[1:] != us[:-1]])
        ends = np.r_[starts[1:], len(us)]
        seg_nodes = np.zeros((128, SP), np.int16)
        v_slots = np.zeros((128, SP, SLOT), np.int16)
        t_slots = np.zeros((128, SP, SLOT), np.float32)
        counts = np.zeros(128, np.int64)
        i = 0
        for s0, e0 in zip(starts, ends):
            n = us[s0]
            for j in range(s0, e0, SLOT):
                p = i % 128
                q = counts[p]
                counts[p] += 1
                assert q < SP, "segment overflow; raise SP"
                i += 1
                seg_nodes[p, q] = n
                va = vs[j:min(j + SLOT, e0)]
                ta = ts[j:min(j + SLOT, e0)]
                v_slots[p, q, :] = n
                v_slots[p, q, : len(va)] = va
                t_slots[p, q, : len(ta)] = ta
        return seg_nodes, v_slots.reshape(128, C_EV), t_slots.reshape(128, C_EV)

    def wrap16(x, nops, per_op):
        # [nops*per_op] index list -> [128, nops, per_op//16]: op ch's
        # index k at [k % 16, ch, k // 16], replicated down 8 blocks
        w = x.reshape(nops, per_op // 16, 16).transpose(2, 0, 1)
        return np.ascontiguousarray(np.tile(w, (8, 1, 1)))

    ident_np = np.eye(128, dtype=np.float32)
    in_maps = []
    for k in range(NCORES):
        sl = slice(k * EV_PER_CORE, (k + 1) * EV_PER_CORE)
        seg_nodes, v_slots, t_slots = pack_events(u_all[sl], v_all[sl], tt[sl])
        # seg gather list position m = q*128 + p -> [SP,128] flat
        seg_list = seg_nodes.T.reshape(-1)
        # v gather chunk ch, list position mm = cc*128 + p over its 32 cols
        v_list = (
            v_slots.reshape(128, EV_CHUNKS, EV_CC)
            .transpose(1, 2, 0)
            .reshape(-1)
        )
        in_maps.append(
            {
                "zv_all": zv,
                "zv_pad": zv_pad,
                "zv_i": np.ascontiguousarray(zv[k * 256:(k + 1) * 256]),
                "ev_u": wrap16(seg_list, SEG_OPS, SEG_PER_OP),
                "ev_v": wrap16(v_list, EV_CHUNKS, EV_PER_CHUNK),
                "ev_t": np.ascontiguousarray(t_slots),
                "tb": tb,
                "t2b": t2b,
                "ident": ident_np,
            }
        )
    return in_maps, (float(t0), float(tn), E)


def _np_event_partial(m, zv):
    """Reference (numpy, f64) per-partition event distance sums for one
    core's marshalled inputs — used by the dev test harnesses."""
    seg_list = np.concatenate(
        [m["ev_u"][:16, so, :].T.reshape(-1) for so in range(SEG_OPS)]
    )
    seg_nodes = seg_list.reshape(SP, 128).T
    v_slots = np.zeros((128, C_EV), np.int64)
    for ch in range(EV_CHUNKS):
        vc = m["ev_v"][:16, ch, :].T.reshape(-1)
        v_slots[:, ch * EV_CC:(ch + 1) * EV_CC] = vc.reshape(EV_CC, 128).T
    t_slots = m["ev_t"].astype(np.float64)
    un = np.repeat(seg_nodes.astype(np.int64), SLOT, axis=1)
    a = zv[un]
    b = zv[v_slots]
    dx = (a[..., 0] - b[..., 0]) + (a[..., 2] - b[..., 2]) * t_slots
    dy = (a[..., 1] - b[..., 1]) + (a[..., 3] - b[..., 3]) * t_slots
    return np.sqrt(dx * dx + dy * dy).sum(axis=1)


def _combine(core_outs, beta, t0, tn, E):
    """core_outs: list of [128, 24] float32 partial-sum tensors."""
    exp_sum = 0.0
    ev_sum = 0.0
    for o in core_outs:
        o = np.asarray(o, dtype=np.float64)
        exp_sum += o[:, 0 : ITILES * S].sum()
        ev_sum += o[:, 20].sum()
    b = float(beta)
    dt = (tn - t0) / S
    event_intensity = E * b - ev_sum
    non_event = np.exp(b) * (exp_sum - S * N) / 2.0 * dt
    return np.float32(event_intensity - 1.0 * non_event)


def kernel(**inputs) -> np.ndarray:
    from concourse.bass_utils import run_bass_kernel_spmd

    nc = _build()
    in_maps, (t0, tn, E) = _marshal(inputs)
    res = run_bass_kernel_spmd(nc, in_maps, core_ids=list(range(NCORES)))
    beta = float(np.asarray(inputs["beta"]).reshape(-1)[0])
    out = _combine([r["out_p"] for r in res.results], beta, t0, tn, E)
    return np.asarray(out, dtype=np.float32)

